# revision 13
# baseline (speedup 1.0000x reference)
"""Trainium2 Bass kernel for nn_MultiHeadCrossAttention (B=32, Nc=2048, H=8, topk=12).

D4 design: single-fp16-term S matmuls; ACT drains S chunks to fp16 SBUF;
DVE pair-max + 3-level tournament -> T[1024]; fp32 pack pm = q10*1024 + slot;
quarter max8 + match_replace rounds extract top-12 with indices; leaf/parity
resolved via group-redundant indirect_copy + mask-reduce; V never materialized:
gather comp columns (fp32), weight, scatter_add into M[(e,r)], finish with 8
accumulated matmuls Wv_e^T @ M_e, diagonal-extract via hrep mask.
"""

import sys
import numpy as np

for p in ("/opt/trn_rl_repo",):
    if p not in sys.path:
        sys.path.insert(0, p)

import ml_dtypes

B, CORES, BPC = 32, 8, 4
H, HD, NQ, TK, C, NC = 8, 16, 16, 12, 128, 2048
NJ = 8 * NC
MAGIC = 12582912.0          # 2^23 + 2^22
BIG = 3.0 * 2.0**32         # MAGIC * 1024: rounds fp32 to multiples of 1024
PACK_MUL = 8192.0 * 1024.0  # value quantum 1/8192 over range (-1, 1)
PACK_ADD = (8192.0 + MAGIC) * 1024.0
PACK_SUB = BIG
LQ_SCALE = 8192.0
LQ_BIAS = 8192.0 + MAGIC
NEG = -1e30

_prog_cache = {}


def _build_program():
    import concourse.bass as bass
    import concourse.mybir as mybir
    import concourse.tile as tile
    from concourse import bacc
    from concourse import library_config

    dt = mybir.dt
    Alu = mybir.AluOpType
    Act = mybir.ActivationFunctionType
    f32, f16, bf16 = dt.float32, dt.float16, dt.bfloat16
    nc = bacc.Bacc("TRN2", target_bir_lowering=False)

    c16h_d = nc.dram_tensor("c16h", [BPC, C, NC], f16, kind="ExternalInput")
    xT_d = nc.dram_tensor("xT", [C, BPC], f32, kind="ExternalInput")
    wq_d = nc.dram_tensor("wq", [C, 2048], f16, kind="ExternalInput")
    xT16_d = nc.dram_tensor("xT16", [C, BPC], f16, kind="ExternalInput")
    wkT_d = nc.dram_tensor("wkT", [C, 8 * C], f16, kind="ExternalInput")
    wv_d = nc.dram_tensor("wv", [C, 8 * C], f16, kind="ExternalInput")
    wjwp_d = nc.dram_tensor("wjwp", [C, NQ * C], f16, kind="ExternalInput")
    wp_d = nc.dram_tensor("wp", [C, C], f32, kind="ExternalInput")
    bp4_d = nc.dram_tensor("bp4", [BPC, C], f32, kind="ExternalInput")
    hrep_d = nc.dram_tensor("hrep", [C, C], f16, kind="ExternalInput")
    me_d = nc.dram_tensor("me", [C, 512], f16, kind="ExternalInput")
    mo_d = nc.dram_tensor("mo", [C, 512], f16, kind="ExternalInput")
    srow_d = nc.dram_tensor("srow", [C, 1024], f16, kind="ExternalInput")
    krow_d = nc.dram_tensor("krow", [C, 96], f16, kind="ExternalInput")
    kval_d = nc.dram_tensor("kval", [C, 192], f16, kind="ExternalInput")
    dsel_d = nc.dram_tensor("dsel", [C, 3072], f16, kind="ExternalInput")
    out_d = nc.dram_tensor("out", [BPC, C], f32, kind="ExternalOutput")

    with tile.TileContext(nc) as tc:
        nc.gpsimd.load_library(library_config.ap_gather)
        with (
            tc.tile_pool(name="weights", bufs=1) as wpool,
            tc.tile_pool(name="inb", bufs=2) as inpool,       # c16h, c32
            tc.tile_pool(name="sbig", bufs=2) as sbig,        # R
            tc.tile_pool(name="sbe1", bufs=1) as sbe1,        # SbEven
            tc.tile_pool(name="stage", bufs=2) as stpool,     # odd chunks
            tc.tile_pool(name="mid", bufs=1) as midpool,      # L2, pm
            tc.tile_pool(name="vt", bufs=2) as vtpool,        # V table
            tc.tile_pool(name="mid2", bufs=2) as mid2pool,    # L1, g1
            tc.tile_pool(name="small", bufs=2) as smpool,     # winner stage tiles
            tc.tile_pool(name="ps_s", bufs=3, space="PSUM") as ps_s,
            tc.tile_pool(name="ps_a", bufs=1, space="PSUM") as ps_a,
            tc.tile_pool(name="ps_m", bufs=1, space="PSUM") as ps_m,
        ):
            # ---- resident weights/constants ----
            wq_s = wpool.tile([C, 2048], f16)
            nc.sync.dma_start(wq_s[:], wq_d[:])
            xT16_s = wpool.tile([C, BPC], f16)
            nc.sync.dma_start(xT16_s[:], xT16_d[:])
            wkT_s = wpool.tile([C, 8 * C], f16)
            nc.sync.dma_start(wkT_s[:], wkT_d[:])
            wv_s = wpool.tile([C, 8 * C], f16)
            nc.sync.dma_start(wv_s[:], wv_d[:])
            wjwp_s = wpool.tile([C, NQ * C], f16)
            nc.sync.dma_start(wjwp_s[:], wjwp_d[:])
            wp_s = wpool.tile([C, C], f32)
            nc.sync.dma_start(wp_s[:], wp_d[:])
            bp4_s = wpool.tile([BPC, C], f32)
            nc.sync.dma_start(bp4_s[:], bp4_d[:])
            hrep_s = wpool.tile([C, C], f16)
            nc.sync.dma_start(hrep_s[:], hrep_d[:])
            me_s = wpool.tile([C, 512], f16)
            nc.sync.dma_start(me_s[:], me_d[:])
            mo_s = wpool.tile([C, 512], f16)
            nc.sync.dma_start(mo_s[:], mo_d[:])
            srow_s = wpool.tile([C, 1024], f16)
            nc.sync.dma_start(srow_s[:], srow_d[:])
            krow_s = wpool.tile([C, 96], f16)
            nc.sync.dma_start(krow_s[:], krow_d[:])
            kval_s = wpool.tile([C, 192], f16)
            nc.sync.dma_start(kval_s[:], kval_d[:])
            dsel_s = wpool.tile([C, 3072], f16)
            nc.sync.dma_start(dsel_s[:], dsel_d[:])
            xT_s = wpool.tile([C, BPC], f32)
            nc.sync.dma_start(xT_s[:], xT_d[:])

            bm1_s = wpool.tile([C, 1], f32)      # exp bias constant
            nc.vector.memset(bm1_s[:], -1.0)

            # ---- Q projection for all batches: qt [(h,hd), (q,b)] ----
            qt_ps = ps_m.tile([C, 512], f32, tag="misc")
            for qi in range(NQ):
                nc.tensor.matmul(
                    qt_ps[:, qi * BPC:(qi + 1) * BPC],
                    wq_s[:, qi * C:(qi + 1) * C],
                    xT16_s[:],
                )
            qt_s = wpool.tile([C, NQ * BPC], f32)
            nc.scalar.copy(qt_s[:], qt_ps[:, : NQ * BPC])

            pvt4_s = wpool.tile([C, NQ * BPC], f16)

            for b in range(BPC):
                c16 = inpool.tile([C, NC], f16, tag="c16")
                nc.sync.dma_start(c16[:], c16h_d[b])

                # ---- qbd (block diag, 0.25 scale), fp16 ----
                qfull_s = smpool.tile([C, C], f16, tag="qfull")
                qsl = (
                    qt_s[:, b::BPC]
                    .rearrange("p (o q) -> p o q", o=1)
                    .to_broadcast([C, H, NQ])
                )
                nc.vector.tensor_scalar(
                    qfull_s[:].rearrange("p (o q) -> p o q", o=H),
                    qsl, 0.25, None, Alu.mult,
                )
                qbd_s = smpool.tile([C, C], f16, tag="qbd")
                nc.vector.tensor_mul(qbd_s[:], qfull_s[:], hrep_s[:])

                # ---- A_e [c, row] fp16 ----
                a16 = stpool.tile([C, 8 * C], f16, tag="a16")
                for half in range(2):
                    a_ps = ps_a.tile([C, 512], f32, tag="a")
                    for i in range(4):
                        e = half * 4 + i
                        nc.tensor.matmul(
                            a_ps[:, i * C:(i + 1) * C],
                            wkT_s[:, e * C:(e + 1) * C],
                            qbd_s[:],
                        )
                    nc.scalar.copy(a16[:, half * 512:(half + 1) * 512], a_ps[:])

                # ---- S chunks -> ACT fp16 copies; L0 pair-max -> R ----
                sbe = sbe1.tile([C, 8192], f16, tag="sbe")   # even chunks
                r_s = sbig.tile([C, 8192], f16, tag="R")
                for k in range(8):   # chunk pair k: chunks 2k (even), 2k+1 (odd)
                    odd = stpool.tile([C, 1024], f16, tag="odd")
                    for ch in (2 * k, 2 * k + 1):
                        e, half = ch // 2, ch % 2
                        s_ps = ps_s.tile([C, 1024], f32, tag="s")
                        for n2 in range(2):
                            col = half * 1024 + n2 * 512
                            nc.tensor.matmul(
                                s_ps[:, n2 * 512:(n2 + 1) * 512],
                                a16[:, e * C:(e + 1) * C],
                                c16[:, col:col + 512],
                            )
                        dst = sbe[:, k * 1024:(k + 1) * 1024] if ch % 2 == 0 else odd[:]
                        nc.scalar.copy(dst, s_ps[:])
                    nc.vector.tensor_max(
                        r_s[:, k * 1024:(k + 1) * 1024],
                        sbe[:, k * 1024:(k + 1) * 1024],
                        odd[:],
                    )

                # ---- V table: vt[(h,hd), j] bf16 ----
                vt_s = vtpool.tile([C, NJ], bf16, tag="VT")
                for ch in range(16):
                    e, half = ch // 2, ch % 2
                    v_ps = ps_s.tile([C, 1024], f32, tag="s")
                    for n2 in range(2):
                        col = half * 1024 + n2 * 512
                        nc.tensor.matmul(
                            v_ps[:, n2 * 512:(n2 + 1) * 512],
                            wv_s[:, e * C:(e + 1) * C],
                            c16[:, col:col + 512],
                        )
                    dst = vt_s[:, ch * 1024:(ch + 1) * 1024]
                    if ch < 12:
                        nc.scalar.copy(dst, v_ps[:])
                    else:
                        nc.vector.tensor_copy(dst, v_ps[:])

                # ---- tournament: R [p,8,1024] -> T [p,1024] ----
                l1_s = mid2pool.tile([C, 4096], f16, tag="L1")
                rv = r_s[:].rearrange("p (k f) -> p k f", k=8)
                nc.vector.tensor_max(
                    l1_s[:].rearrange("p (k f) -> p k f", k=4),
                    rv[:, 0:4, :], rv[:, 4:8, :],
                )
                l2_s = midpool.tile([C, 2048], f16, tag="L2")
                l1v = l1_s[:].rearrange("p (k f) -> p k f", k=4)
                nc.vector.tensor_max(
                    l2_s[:].rearrange("p (k f) -> p k f", k=2),
                    l1v[:, 0:2, :], l1v[:, 2:4, :],
                )
                t_s = midpool.tile([C, 1024], f16, tag="T")
                nc.vector.tensor_max(t_s[:], l2_s[:, 0:1024], l2_s[:, 1024:2048])

                # ---- pack pm = q10*1024 + s (exact fp32 ints) ----
                t1_s = midpool.tile([C, 1024], f32, tag="t1")
                nc.vector.tensor_scalar(
                    t1_s[:], t_s[:], PACK_MUL, PACK_ADD, Alu.mult, Alu.add
                )
                pm_s = midpool.tile([C, 1024], f32, tag="pm")
                nc.vector.scalar_tensor_tensor(
                    pm_s[:], t1_s[:], PACK_SUB, srow_s[:], Alu.subtract, Alu.add
                )

                # ---- quarter extract -> 32 cands -> top8 + next4 ----
                cand_s = smpool.tile([C, 32], f32, tag="cand")
                for qd in range(4):
                    nc.vector.max(
                        cand_s[:, qd * 8:(qd + 1) * 8],
                        pm_s[:, qd * 256:(qd + 1) * 256],
                    )
                t8a = smpool.tile([C, 8], f32, tag="t8a")
                nc.vector.max(t8a[:], cand_s[:])
                c2_s = smpool.tile([C, 32], f32, tag="c2")
                nc.vector.match_replace(c2_s[:], t8a[:], cand_s[:], NEG)
                t8b = smpool.tile([C, 8], f32, tag="t8b")
                nc.vector.max(t8b[:], c2_s[:])
                pw_s = smpool.tile([C, 12], f32, tag="pw")
                nc.vector.tensor_copy(pw_s[:, 0:8], t8a[:])
                nc.vector.tensor_copy(pw_s[:, 8:12], t8b[:, 0:4])

                # ---- decode: r1 = round1024(pm); s = pm - r1 (mod fix); qv ----
                r1_s = smpool.tile([C, 12], f32, tag="r1")
                nc.vector.tensor_scalar(
                    r1_s[:], pw_s[:], BIG, BIG, Alu.add, Alu.subtract
                )
                sp_s = smpool.tile([C, 12], f32, tag="sp")
                nc.vector.tensor_sub(sp_s[:], pw_s[:], r1_s[:])
                neg_s = smpool.tile([C, 12], f32, tag="neg")
                nc.vector.tensor_scalar(neg_s[:], sp_s[:], 0.0, None, Alu.is_lt)
                s_sl = smpool.tile([C, 12], f32, tag="s")
                nc.vector.scalar_tensor_tensor(
                    s_sl[:], neg_s[:], 1024.0, sp_s[:], Alu.mult, Alu.add
                )
                qv_s = smpool.tile([C, 12], f32, tag="qv")
                nc.vector.scalar_tensor_tensor(
                    qv_s[:], r1_s[:], 1.0 / 1024.0, neg_s[:], Alu.mult, Alu.subtract
                )
                g0_s = smpool.tile([C, 12], f32, tag="g0")
                nc.vector.tensor_scalar(g0_s[:], qv_s[:], MAGIC, None, Alu.add)

                # ---- softmax weights from quantized values ----
                expv_s = smpool.tile([C, 12], f32, tag="expv")
                nc.scalar.activation(
                    expv_s[:], qv_s[:], Act.Exp, bias=bm1_s[:], scale=1.0 / 8192.0
                )
                den_s = smpool.tile([C, 1], f32, tag="den")
                nc.vector.tensor_reduce(
                    den_s[:], expv_s[:], mybir.AxisListType.X, Alu.add
                )
                rden_s = smpool.tile([C, 1], f32, tag="rden")
                nc.vector.reciprocal(rden_s[:], den_s[:])

                # ---- leaf resolve: ap_gather 8 leaf-pairs/winner (group lists) ----
                s2_s = smpool.tile([C, 12], f32, tag="s2")
                nc.vector.tensor_scalar(
                    s2_s[:], s_sl[:], 0.5, -0.25, Alu.mult, Alu.add
                )
                nc.vector.tensor_scalar(
                    s2_s[:], s2_s[:], MAGIC, MAGIC, Alu.add, Alu.subtract
                )
                pb_s = smpool.tile([C, 12], f32, tag="pb")
                nc.vector.scalar_tensor_tensor(
                    pb_s[:], s2_s[:], -2.0, s_sl[:], Alu.mult, Alu.add
                )
                i1_s = midpool.tile([C, 96], f32, tag="i1")
                nc.vector.tensor_add(
                    i1_s[:].rearrange("p (w k) -> p w k", w=12),
                    krow_s[:].rearrange("p (w k) -> p w k", w=12),
                    s2_s[:].rearrange("p (w o) -> p w o", o=1).to_broadcast([C, 12, 8]),
                )
                i1u_s = smpool.tile([C, 96], dt.int16, tag="i1u")
                nc.vector.tensor_copy(i1u_s[:], i1_s[:])
                g1_s = midpool.tile([C, 3072], f16, tag="g1")
                nc.gpsimd.ap_gather(
                    g1_s[:], r_s[:], i1u_s[:],
                    channels=C, num_elems=4096, d=2, num_idxs=1536,
                )
                nc.vector.tensor_mul(g1_s[:], g1_s[:], dsel_s[:])
                diag_s = midpool.tile([C, 192], f32, tag="diag")
                nc.vector.tensor_reduce(
                    diag_s[:],
                    g1_s[:].rearrange("p (t b pr) -> p t pr b", t=96, b=16),
                    mybir.AxisListType.X, Alu.add,
                )
                lq_s = midpool.tile([C, 192], f32, tag="lq")
                nc.vector.tensor_scalar(
                    lq_s[:], diag_s[:], LQ_SCALE, LQ_BIAS, Alu.mult, Alu.add
                )
                eq_s = midpool.tile([C, 192], f32, tag="eq")
                nc.vector.tensor_tensor(
                    eq_s[:].rearrange("p (w k) -> p w k", w=12),
                    lq_s[:].rearrange("p (w k) -> p w k", w=12),
                    g0_s[:].rearrange("p (w o) -> p w o", o=1).to_broadcast([C, 12, 16]),
                    Alu.is_equal,
                )
                kk_s = midpool.tile([C, 192], f32, tag="kk")
                nc.vector.tensor_mul(kk_s[:], eq_s[:], kval_s[:])
                kp1_s = smpool.tile([C, 12], f32, tag="kp1")
                nc.vector.tensor_reduce(
                    kp1_s[:],
                    kk_s[:].rearrange("p (w k) -> p w k", w=12),
                    mybir.AxisListType.X, Alu.max,
                )

                # ---- chunk parity: gather even-chunk pair, pick element ----
                i2_s = smpool.tile([C, 12], f32, tag="i2")
                nc.vector.tensor_scalar(
                    i2_s[:], kp1_s[:], 512.0, -512.0, Alu.mult, Alu.add
                )
                nc.vector.tensor_add(i2_s[:], i2_s[:], s2_s[:])
                i2u_s = smpool.tile([C, 12], dt.int16, tag="i2u")
                nc.vector.tensor_copy(i2u_s[:], i2_s[:])
                g2_s = midpool.tile([C, 384], f16, tag="g2")
                nc.gpsimd.ap_gather(
                    g2_s[:], sbe[:], i2u_s[:],
                    channels=C, num_elems=4096, d=2, num_idxs=192,
                )
                g2m_s = midpool.tile([C, 384], f16, tag="g2m")
                nc.vector.tensor_mul(g2m_s[:], g2_s[:], dsel_s[:, 0:384])
                dpair_s = smpool.tile([C, 24], f32, tag="dpair")
                nc.vector.tensor_reduce(
                    dpair_s[:],
                    g2m_s[:].rearrange("p (w b pr) -> p w pr b", w=12, b=16),
                    mybir.AxisListType.X, Alu.add,
                )
                evd_s = smpool.tile([C, 12], f32, tag="evd")
                nc.vector.tensor_sub(
                    evd_s[:], dpair_s[:, 1::2], dpair_s[:, 0::2]
                )
                ev_s = smpool.tile([C, 12], f32, tag="ev")
                nc.vector.scalar_tensor_tensor(
                    ev_s[:], pb_s[:], 1.0, evd_s[:], Alu.mult, Alu.mult
                )
                nc.vector.tensor_add(ev_s[:], ev_s[:], dpair_s[:, 0::2])
                evq_s = smpool.tile([C, 12], f32, tag="evq")
                nc.vector.tensor_scalar(
                    evq_s[:], ev_s[:], LQ_SCALE, LQ_BIAS, Alu.mult, Alu.add
                )
                par_s = smpool.tile([C, 12], f32, tag="par")
                nc.vector.tensor_tensor(
                    par_s[:], evq_s[:], g0_s[:], Alu.not_equal
                )

                # ---- j (global winner index) ----
                j0_s = smpool.tile([C, 12], f32, tag="j0")
                nc.vector.scalar_tensor_tensor(
                    j0_s[:], par_s[:], 1024.0, s_sl[:], Alu.mult, Alu.add
                )
                jk_s = smpool.tile([C, 12], f32, tag="jk")
                nc.vector.tensor_scalar(
                    jk_s[:], kp1_s[:], 2048.0, -2048.0, Alu.mult, Alu.add
                )
                j_s = smpool.tile([C, 12], f32, tag="j")
                nc.vector.tensor_add(j_s[:], j0_s[:], jk_s[:])

                # ---- pair idx gp = floor(j/2), pair parity; padded to 16 ----
                gp_s = smpool.tile([C, 16], f32, tag="gp")
                nc.vector.memset(gp_s[:], 0.0)
                nc.vector.tensor_scalar(
                    gp_s[:, 0:12], j_s[:], 0.5, -0.25, Alu.mult, Alu.add
                )
                nc.vector.tensor_scalar(
                    gp_s[:, 0:12], gp_s[:, 0:12], MAGIC, MAGIC, Alu.add, Alu.subtract
                )
                gp_i = smpool.tile([C, 16], dt.int16, tag="gpi")
                nc.vector.tensor_copy(gp_i[:], gp_s[:])
                par2_s = smpool.tile([C, 16], f32, tag="par2")
                nc.vector.memset(par2_s[:], 0.0)
                nc.vector.scalar_tensor_tensor(
                    par2_s[:, 0:12], gp_s[:, 0:12], -2.0, j_s[:], Alu.mult, Alu.add
                )
                wn_s = smpool.tile([C, 16], f32, tag="wn")
                nc.vector.memset(wn_s[:], 0.0)
                nc.vector.tensor_scalar(
                    wn_s[:, 0:12], expv_s[:], rden_s[:], None, Alu.mult
                )
                wnE_s = smpool.tile([C, 16], f32, tag="wnE")
                nc.vector.scalar_tensor_tensor(
                    wnE_s[:], par2_s[:], -1.0, wn_s[:], Alu.mult, Alu.mult
                )
                nc.vector.tensor_add(wnE_s[:], wnE_s[:], wn_s[:])
                wnO_s = smpool.tile([C, 16], f32, tag="wnO")
                nc.vector.tensor_mul(wnO_s[:], wn_s[:], par2_s[:])

                # ---- gather V pairs (per-head core lists) ----
                g_s = smpool.tile([C, 512], bf16, tag="G")
                nc.gpsimd.ap_gather(
                    g_s[:], vt_s[:], gp_i[:],
                    channels=C, num_elems=NJ // 2, d=2, num_idxs=256,
                )

                # ---- weights -> [(h,d), (i,q,parity)] via headrep matmul ----
                wEb = (
                    wnE_s[:].rearrange("p (i o) -> p i o", o=1)
                    .to_broadcast([C, NQ, 32])
                )
                wOb = (
                    wnO_s[:].rearrange("p (i o) -> p i o", o=1)
                    .to_broadcast([C, NQ, 32])
                )
                tmpE = midpool.tile([C, 512], f16, tag="tmpE")
                nc.vector.tensor_mul(
                    tmpE[:].rearrange("p (i s) -> p i s", s=32),
                    wEb,
                    me_s[:].rearrange("p (i s) -> p i s", s=32),
                )
                wsc = midpool.tile([C, 512], f16, tag="wsc")
                nc.vector.tensor_mul(
                    wsc[:].rearrange("p (i s) -> p i s", s=32),
                    wOb,
                    mo_s[:].rearrange("p (i s) -> p i s", s=32),
                )
                nc.vector.tensor_add(wsc[:], wsc[:], tmpE[:])
                wb_ps = ps_m.tile([C, 512], f32, tag="misc")
                nc.tensor.matmul(wb_ps[:], hrep_s[:], wsc[:])
                wb_s = smpool.tile([C, 512], bf16, tag="wb")
                nc.scalar.copy(wb_s[:], wb_ps[:])

                gw_s = midpool.tile([C, 512], f32, tag="gw")
                nc.vector.tensor_mul(gw_s[:], g_s[:], wb_s[:])
                with nc.allow_low_precision(reason="pvt4 f16 out, values tiny"):
                    nc.vector.tensor_reduce(
                        pvt4_s[:, b::BPC],
                        gw_s[:].rearrange("p (i q r) -> p q i r", q=NQ, r=2),
                        mybir.AxisListType.XY,
                        Alu.add,
                    )

            # ---- final projections ----
            o1_ps = ps_m.tile([C, 512], f32, tag="misc")
            for qi in range(NQ):
                nc.tensor.matmul(
                    o1_ps[:, 0:BPC],
                    wjwp_s[:, qi * C:(qi + 1) * C],
                    pvt4_s[:, qi * BPC:(qi + 1) * BPC],
                    start=(qi == 0),
                    stop=(qi == NQ - 1),
                )
            o2_s = smpool.tile([C, BPC], f32, tag="o2")
            nc.vector.tensor_add(o2_s[:], o1_ps[:, 0:BPC], xT_s[:])
            o3_ps = ps_m.tile([C, 512], f32, tag="misc")
            nc.tensor.matmul(o3_ps[0:BPC, 0:C], o2_s[:], wp_s[:])
            o4_s = smpool.tile([BPC, C], f32, tag="o4")
            nc.vector.tensor_add(o4_s[:], o3_ps[0:BPC, 0:C], bp4_s[:])
            nc.sync.dma_start(out_d[:], o4_s[:])

    nc.compile()
    return nc


def _host_prep(inputs):
    x = np.asarray(inputs["x"], dtype=np.float32)
    complement = np.asarray(inputs["complement"], np.float32)
    Wq = np.asarray(inputs["Wq"], np.float32)
    Wkv = np.asarray(inputs["Wkv"], np.float32)
    Wjw = np.asarray(inputs["Wjw"], np.float32)
    Wp = np.asarray(inputs["Wp"], np.float32)
    bp = np.asarray(inputs["bp"], np.float32)

    wkT = np.empty((C, 8 * C), np.float32)
    wv = np.empty((C, 8 * C), np.float32)
    for e in range(8):
        wkT[:, e * C:(e + 1) * C] = Wkv[:, e * 256: e * 256 + 128].T
        wv[:, e * C:(e + 1) * C] = Wkv[:, e * 256 + 128: e * 256 + 256]
    wjwp = (
        Wjw.reshape(H, NQ, HD, C).transpose(1, 0, 2, 3).reshape(NQ, C, C)
        .transpose(1, 0, 2).reshape(C, NQ * C)
    )
    bp4 = np.tile(bp.reshape(1, C), (BPC, 1)).astype(np.float32)
    hrep = np.kron(np.eye(H, dtype=np.float32), np.ones((HD, HD), np.float32))
    s_idx = np.tile(np.arange(32).reshape(1, 1, 32), (C, NQ, 1))
    p_idx = (np.arange(C) % NQ).reshape(C, 1, 1)
    me = (s_idx == 2 * p_idx).astype(np.float32).reshape(C, 512)
    mo = (s_idx == 2 * p_idx + 1).astype(np.float32).reshape(C, 512)
    srow = np.tile(np.arange(1024, dtype=np.float32).reshape(1, 1024), (C, 1))
    krow = np.tile(
        (np.tile(np.arange(8, dtype=np.float32), 12) * 512).reshape(1, 96), (C, 1)
    )
    kval = np.repeat(
        np.tile(
            (np.tile(np.arange(8, dtype=np.float32), 12) + 1).reshape(1, 96), (C, 1)
        ), 2, axis=1,
    )
    dsel1 = np.zeros((C, 1536), np.float16)
    for p in range(C):
        dsel1[p, (np.arange(96) * 16 + p % 16)] = 1.0
    dsel = np.repeat(dsel1, 2, axis=1)

    shared = dict(
        wq=np.ascontiguousarray(Wq.astype(np.float16)),
        wkT=np.ascontiguousarray(wkT.astype(np.float16)),
        wv=np.ascontiguousarray(wv.astype(np.float16)),
        wjwp=np.ascontiguousarray(wjwp.astype(np.float16)),
        wp=np.ascontiguousarray(Wp),
        bp4=bp4,
        hrep=np.ascontiguousarray(hrep.astype(np.float16)),
        me=np.ascontiguousarray(me.astype(np.float16)),
        mo=np.ascontiguousarray(mo.astype(np.float16)),
        srow=srow.astype(np.float16),
        krow=np.ascontiguousarray(krow.astype(np.float16)),
        kval=np.ascontiguousarray(kval.astype(np.float16)),
        dsel=dsel,
    )

    in_maps = []
    for core in range(CORES):
        bs = range(core * BPC, (core + 1) * BPC)
        comp = np.stack(
            [
                np.concatenate([x[bb].reshape(1, C), complement[bb]], axis=0)
                for bb in bs
            ]
        ).astype(np.float32)
        compT = comp.transpose(0, 2, 1)
        m = dict(shared)
        m["c16h"] = np.ascontiguousarray(compT.astype(np.float16))
        xTc = np.ascontiguousarray(x[list(bs)].reshape(BPC, C).T)
        m["xT"] = xTc
        m["xT16"] = xTc.astype(np.float16)
        in_maps.append(m)
    return in_maps


def kernel(**inputs):
    from concourse.bass_utils import run_bass_kernel_spmd

    if "prog" not in _prog_cache:
        _prog_cache["prog"] = _build_program()
    nc = _prog_cache["prog"]

    in_maps = _host_prep(inputs)
    res = run_bass_kernel_spmd(nc, in_maps, core_ids=list(range(CORES)))
    out = np.empty((B, 1, C), np.float32)
    for core in range(CORES):
        o = res.results[core]["out"]
        for i in range(BPC):
            out[core * BPC + i, 0, :] = o[i]
    return out


if __name__ == "__main__":
    d = np.load("/root/problem/inputs_cache.npz")
    inputs = {k: d[k] for k in d.files if k != "ref_out"}
    ref = d["ref_out"]
    got = kernel(**inputs)
    err = np.abs(got - ref)
    print("absmax err:", err.max())
    print("Relative error:", err.max() / np.abs(ref).max())
    print("rel l2:", np.linalg.norm(got - ref) / np.linalg.norm(ref))


# revision 14
# speedup vs baseline: 1.0057x; 1.0057x over previous
"""Trainium2 Bass kernel for nn_MultiHeadCrossAttention (B=32, Nc=2048, H=8, topk=12).

D4 design: single-fp16-term S matmuls; ACT drains S chunks to fp16 SBUF;
DVE pair-max + 3-level tournament -> T[1024]; fp32 pack pm = q10*1024 + slot;
quarter max8 + match_replace rounds extract top-12 with indices; leaf/parity
resolved via group-redundant indirect_copy + mask-reduce; V never materialized:
gather comp columns (fp32), weight, scatter_add into M[(e,r)], finish with 8
accumulated matmuls Wv_e^T @ M_e, diagonal-extract via hrep mask.
"""

import sys
import numpy as np

for p in ("/opt/trn_rl_repo",):
    if p not in sys.path:
        sys.path.insert(0, p)

import ml_dtypes

B, CORES, BPC = 32, 8, 4
H, HD, NQ, TK, C, NC = 8, 16, 16, 12, 128, 2048
NJ = 8 * NC
MAGIC = 12582912.0          # 2^23 + 2^22
BIG = 3.0 * 2.0**32         # MAGIC * 1024: rounds fp32 to multiples of 1024
PACK_MUL = 8192.0 * 1024.0  # value quantum 1/8192 over range (-1, 1)
PACK_ADD = (8192.0 + MAGIC) * 1024.0
PACK_SUB = BIG
LQ_SCALE = 8192.0
LQ_BIAS = 8192.0 + MAGIC
NEG = -1e30

_prog_cache = {}


def _build_program():
    import concourse.bass as bass
    import concourse.mybir as mybir
    import concourse.tile as tile
    from concourse import bacc
    from concourse import library_config

    dt = mybir.dt
    Alu = mybir.AluOpType
    Act = mybir.ActivationFunctionType
    f32, f16, bf16 = dt.float32, dt.float16, dt.bfloat16
    nc = bacc.Bacc("TRN2", target_bir_lowering=False)

    c16h_d = nc.dram_tensor("c16h", [BPC, C, NC], f16, kind="ExternalInput")
    xT_d = nc.dram_tensor("xT", [C, BPC], f32, kind="ExternalInput")
    wq_d = nc.dram_tensor("wq", [C, 2048], f16, kind="ExternalInput")
    xT16_d = nc.dram_tensor("xT16", [C, BPC], f16, kind="ExternalInput")
    wkT_d = nc.dram_tensor("wkT", [C, 8 * C], f16, kind="ExternalInput")
    wv_d = nc.dram_tensor("wv", [C, 8 * C], f16, kind="ExternalInput")
    wjwp_d = nc.dram_tensor("wjwp", [C, NQ * C], f16, kind="ExternalInput")
    wp_d = nc.dram_tensor("wp", [C, C], f32, kind="ExternalInput")
    bp4_d = nc.dram_tensor("bp4", [BPC, C], f32, kind="ExternalInput")
    hrep_d = nc.dram_tensor("hrep", [C, C], f16, kind="ExternalInput")
    me_d = nc.dram_tensor("me", [C, 512], f16, kind="ExternalInput")
    mo_d = nc.dram_tensor("mo", [C, 512], f16, kind="ExternalInput")
    srow_d = nc.dram_tensor("srow", [C, 1024], f16, kind="ExternalInput")
    krow_d = nc.dram_tensor("krow", [C, 96], f16, kind="ExternalInput")
    kval_d = nc.dram_tensor("kval", [C, 192], f16, kind="ExternalInput")
    dsel_d = nc.dram_tensor("dsel", [C, 3072], f16, kind="ExternalInput")
    out_d = nc.dram_tensor("out", [BPC, C], f32, kind="ExternalOutput")

    with tile.TileContext(nc) as tc:
        nc.gpsimd.load_library(library_config.ap_gather)
        with (
            tc.tile_pool(name="weights", bufs=1) as wpool,
            tc.tile_pool(name="inb", bufs=2) as inpool,       # c16h, c32
            tc.tile_pool(name="sbig", bufs=2) as sbig,        # R
            tc.tile_pool(name="sbe1", bufs=2) as sbe1,        # SbEven
            tc.tile_pool(name="stage", bufs=2) as stpool,     # odd chunks
            tc.tile_pool(name="mid", bufs=1) as midpool,      # L2, pm
            tc.tile_pool(name="vt", bufs=1) as vtpool,        # V table
            tc.tile_pool(name="mid2", bufs=2) as mid2pool,    # L1, g1
            tc.tile_pool(name="small", bufs=2) as smpool,     # winner stage tiles
            tc.tile_pool(name="ps_s", bufs=2, space="PSUM") as ps_s,
            tc.tile_pool(name="ps_v", bufs=1, space="PSUM") as ps_v,
            tc.tile_pool(name="ps_a", bufs=1, space="PSUM") as ps_a,
            tc.tile_pool(name="ps_m", bufs=1, space="PSUM") as ps_m,
        ):
            # ---- resident weights/constants ----
            wq_s = wpool.tile([C, 2048], f16)
            nc.sync.dma_start(wq_s[:], wq_d[:])
            xT16_s = wpool.tile([C, BPC], f16)
            nc.sync.dma_start(xT16_s[:], xT16_d[:])
            wkT_s = wpool.tile([C, 8 * C], f16)
            nc.sync.dma_start(wkT_s[:], wkT_d[:])
            wv_s = wpool.tile([C, 8 * C], f16)
            nc.sync.dma_start(wv_s[:], wv_d[:])
            wjwp_s = wpool.tile([C, NQ * C], f16)
            nc.sync.dma_start(wjwp_s[:], wjwp_d[:])
            wp_s = wpool.tile([C, C], f32)
            nc.sync.dma_start(wp_s[:], wp_d[:])
            bp4_s = wpool.tile([BPC, C], f32)
            nc.sync.dma_start(bp4_s[:], bp4_d[:])
            hrep_s = wpool.tile([C, C], f16)
            nc.sync.dma_start(hrep_s[:], hrep_d[:])
            me_s = wpool.tile([C, 512], f16)
            nc.sync.dma_start(me_s[:], me_d[:])
            mo_s = wpool.tile([C, 512], f16)
            nc.sync.dma_start(mo_s[:], mo_d[:])
            srow_s = wpool.tile([C, 1024], f16)
            nc.sync.dma_start(srow_s[:], srow_d[:])
            krow_s = wpool.tile([C, 96], f16)
            nc.sync.dma_start(krow_s[:], krow_d[:])
            kval_s = wpool.tile([C, 192], f16)
            nc.sync.dma_start(kval_s[:], kval_d[:])
            dsel_s = wpool.tile([C, 3072], f16)
            nc.sync.dma_start(dsel_s[:], dsel_d[:])
            xT_s = wpool.tile([C, BPC], f32)
            nc.sync.dma_start(xT_s[:], xT_d[:])

            bm1_s = wpool.tile([C, 1], f32)      # exp bias constant
            nc.vector.memset(bm1_s[:], -1.0)

            # ---- Q projection for all batches: qt [(h,hd), (q,b)] ----
            qt_ps = ps_m.tile([C, 512], f32, tag="misc")
            for qi in range(NQ):
                nc.tensor.matmul(
                    qt_ps[:, qi * BPC:(qi + 1) * BPC],
                    wq_s[:, qi * C:(qi + 1) * C],
                    xT16_s[:],
                )
            qt_s = wpool.tile([C, NQ * BPC], f32)
            nc.scalar.copy(qt_s[:], qt_ps[:, : NQ * BPC])

            pvt4_s = wpool.tile([C, NQ * BPC], f16)

            for b in range(BPC):
                c16 = inpool.tile([C, NC], f16, tag="c16")
                nc.sync.dma_start(c16[:], c16h_d[b])

                # ---- qbd (block diag, 0.25 scale), fp16 ----
                qfull_s = smpool.tile([C, C], f16, tag="qfull")
                qsl = (
                    qt_s[:, b::BPC]
                    .rearrange("p (o q) -> p o q", o=1)
                    .to_broadcast([C, H, NQ])
                )
                nc.vector.tensor_scalar(
                    qfull_s[:].rearrange("p (o q) -> p o q", o=H),
                    qsl, 0.25, None, Alu.mult,
                )
                qbd_s = smpool.tile([C, C], f16, tag="qbd")
                nc.vector.tensor_mul(qbd_s[:], qfull_s[:], hrep_s[:])

                # ---- A_e [c, row] fp16 ----
                a16 = stpool.tile([C, 8 * C], f16, tag="a16")
                for half in range(2):
                    a_ps = ps_a.tile([C, 512], f32, tag="a")
                    for i in range(4):
                        e = half * 4 + i
                        nc.tensor.matmul(
                            a_ps[:, i * C:(i + 1) * C],
                            wkT_s[:, e * C:(e + 1) * C],
                            qbd_s[:],
                        )
                    nc.scalar.copy(a16[:, half * 512:(half + 1) * 512], a_ps[:])

                # ---- S chunks -> ACT fp16 copies; L0 pair-max -> R ----
                sbe = sbe1.tile([C, 8192], f16, tag="sbe")   # even chunks
                r_s = sbig.tile([C, 8192], f16, tag="R")
                for k in range(8):   # chunk pair k: chunks 2k (even), 2k+1 (odd)
                    odd = stpool.tile([C, 1024], f16, tag="odd")
                    for ch in (2 * k, 2 * k + 1):
                        e, half = ch // 2, ch % 2
                        s_ps = ps_s.tile([C, 1024], f32, tag="s")
                        for n2 in range(2):
                            col = half * 1024 + n2 * 512
                            nc.tensor.matmul(
                                s_ps[:, n2 * 512:(n2 + 1) * 512],
                                a16[:, e * C:(e + 1) * C],
                                c16[:, col:col + 512],
                            )
                        dst = sbe[:, k * 1024:(k + 1) * 1024] if ch % 2 == 0 else odd[:]
                        nc.scalar.copy(dst, s_ps[:])
                    nc.vector.tensor_max(
                        r_s[:, k * 1024:(k + 1) * 1024],
                        sbe[:, k * 1024:(k + 1) * 1024],
                        odd[:],
                    )

                # ---- V table: vt[(h,hd), j] bf16 ----
                vt_s = vtpool.tile([C, NJ], bf16, tag="VT")
                for ch in range(16):
                    e, half = ch // 2, ch % 2
                    v_ps = ps_v.tile([C, 1024], f32, tag="v")
                    for n2 in range(2):
                        col = half * 1024 + n2 * 512
                        nc.tensor.matmul(
                            v_ps[:, n2 * 512:(n2 + 1) * 512],
                            wv_s[:, e * C:(e + 1) * C],
                            c16[:, col:col + 512],
                        )
                    dst = vt_s[:, ch * 1024:(ch + 1) * 1024]
                    if ch < 12:
                        nc.scalar.copy(dst, v_ps[:])
                    else:
                        nc.vector.tensor_copy(dst, v_ps[:])

                # ---- tournament: R [p,8,1024] -> T [p,1024] ----
                l1_s = mid2pool.tile([C, 4096], f16, tag="L1")
                rv = r_s[:].rearrange("p (k f) -> p k f", k=8)
                nc.vector.tensor_max(
                    l1_s[:].rearrange("p (k f) -> p k f", k=4),
                    rv[:, 0:4, :], rv[:, 4:8, :],
                )
                l2_s = midpool.tile([C, 2048], f16, tag="L2")
                l1v = l1_s[:].rearrange("p (k f) -> p k f", k=4)
                nc.vector.tensor_max(
                    l2_s[:].rearrange("p (k f) -> p k f", k=2),
                    l1v[:, 0:2, :], l1v[:, 2:4, :],
                )
                t_s = midpool.tile([C, 1024], f16, tag="T")
                nc.vector.tensor_max(t_s[:], l2_s[:, 0:1024], l2_s[:, 1024:2048])

                # ---- pack pm = q10*1024 + s (exact fp32 ints) ----
                t1_s = midpool.tile([C, 1024], f32, tag="t1")
                nc.vector.tensor_scalar(
                    t1_s[:], t_s[:], PACK_MUL, PACK_ADD, Alu.mult, Alu.add
                )
                pm_s = midpool.tile([C, 1024], f32, tag="pm")
                nc.vector.scalar_tensor_tensor(
                    pm_s[:], t1_s[:], PACK_SUB, srow_s[:], Alu.subtract, Alu.add
                )

                # ---- quarter extract -> 32 cands -> top8 + next4 ----
                cand_s = smpool.tile([C, 32], f32, tag="cand")
                for qd in range(4):
                    nc.vector.max(
                        cand_s[:, qd * 8:(qd + 1) * 8],
                        pm_s[:, qd * 256:(qd + 1) * 256],
                    )
                t8a = smpool.tile([C, 8], f32, tag="t8a")
                nc.vector.max(t8a[:], cand_s[:])
                c2_s = smpool.tile([C, 32], f32, tag="c2")
                nc.vector.match_replace(c2_s[:], t8a[:], cand_s[:], NEG)
                t8b = smpool.tile([C, 8], f32, tag="t8b")
                nc.vector.max(t8b[:], c2_s[:])
                pw_s = smpool.tile([C, 12], f32, tag="pw")
                nc.vector.tensor_copy(pw_s[:, 0:8], t8a[:])
                nc.vector.tensor_copy(pw_s[:, 8:12], t8b[:, 0:4])

                # ---- decode: r1 = round1024(pm); s = pm - r1 (mod fix); qv ----
                r1_s = smpool.tile([C, 12], f32, tag="r1")
                nc.vector.tensor_scalar(
                    r1_s[:], pw_s[:], BIG, BIG, Alu.add, Alu.subtract
                )
                sp_s = smpool.tile([C, 12], f32, tag="sp")
                nc.vector.tensor_sub(sp_s[:], pw_s[:], r1_s[:])
                neg_s = smpool.tile([C, 12], f32, tag="neg")
                nc.vector.tensor_scalar(neg_s[:], sp_s[:], 0.0, None, Alu.is_lt)
                s_sl = smpool.tile([C, 12], f32, tag="s")
                nc.vector.scalar_tensor_tensor(
                    s_sl[:], neg_s[:], 1024.0, sp_s[:], Alu.mult, Alu.add
                )
                qv_s = smpool.tile([C, 12], f32, tag="qv")
                nc.vector.scalar_tensor_tensor(
                    qv_s[:], r1_s[:], 1.0 / 1024.0, neg_s[:], Alu.mult, Alu.subtract
                )
                g0_s = smpool.tile([C, 12], f32, tag="g0")
                nc.vector.tensor_scalar(g0_s[:], qv_s[:], MAGIC, None, Alu.add)

                # ---- softmax weights from quantized values ----
                expv_s = smpool.tile([C, 12], f32, tag="expv")
                nc.scalar.activation(
                    expv_s[:], qv_s[:], Act.Exp, bias=bm1_s[:], scale=1.0 / 8192.0
                )
                den_s = smpool.tile([C, 1], f32, tag="den")
                nc.vector.tensor_reduce(
                    den_s[:], expv_s[:], mybir.AxisListType.X, Alu.add
                )
                rden_s = smpool.tile([C, 1], f32, tag="rden")
                nc.vector.reciprocal(rden_s[:], den_s[:])

                # ---- leaf resolve: ap_gather 8 leaf-pairs/winner (group lists) ----
                s2_s = smpool.tile([C, 12], f32, tag="s2")
                nc.vector.tensor_scalar(
                    s2_s[:], s_sl[:], 0.5, -0.25, Alu.mult, Alu.add
                )
                nc.vector.tensor_scalar(
                    s2_s[:], s2_s[:], MAGIC, MAGIC, Alu.add, Alu.subtract
                )
                pb_s = smpool.tile([C, 12], f32, tag="pb")
                nc.vector.scalar_tensor_tensor(
                    pb_s[:], s2_s[:], -2.0, s_sl[:], Alu.mult, Alu.add
                )
                i1_s = midpool.tile([C, 96], f32, tag="i1")
                nc.vector.tensor_add(
                    i1_s[:].rearrange("p (w k) -> p w k", w=12),
                    krow_s[:].rearrange("p (w k) -> p w k", w=12),
                    s2_s[:].rearrange("p (w o) -> p w o", o=1).to_broadcast([C, 12, 8]),
                )
                i1u_s = smpool.tile([C, 96], dt.int16, tag="i1u")
                nc.vector.tensor_copy(i1u_s[:], i1_s[:])
                g1_s = midpool.tile([C, 3072], f16, tag="g1")
                nc.gpsimd.ap_gather(
                    g1_s[:], r_s[:], i1u_s[:],
                    channels=C, num_elems=4096, d=2, num_idxs=1536,
                )
                g1m_s = midpool.tile([C, 3072], f16, tag="g1m")
                nc.vector.tensor_mul(g1m_s[:], g1_s[:], dsel_s[:])
                diag_s = midpool.tile([C, 192], f32, tag="diag")
                nc.vector.tensor_reduce(
                    diag_s[:],
                    g1m_s[:].rearrange("p (t b pr) -> p t pr b", t=96, b=16),
                    mybir.AxisListType.X, Alu.add,
                )
                lq_s = midpool.tile([C, 192], f32, tag="lq")
                nc.vector.tensor_scalar(
                    lq_s[:], diag_s[:], LQ_SCALE, LQ_BIAS, Alu.mult, Alu.add
                )
                eq_s = midpool.tile([C, 192], f32, tag="eq")
                nc.vector.tensor_tensor(
                    eq_s[:].rearrange("p (w k) -> p w k", w=12),
                    lq_s[:].rearrange("p (w k) -> p w k", w=12),
                    g0_s[:].rearrange("p (w o) -> p w o", o=1).to_broadcast([C, 12, 16]),
                    Alu.is_equal,
                )
                kk_s = midpool.tile([C, 192], f32, tag="kk")
                nc.vector.tensor_mul(kk_s[:], eq_s[:], kval_s[:])
                kp1_s = smpool.tile([C, 12], f32, tag="kp1")
                nc.vector.tensor_reduce(
                    kp1_s[:],
                    kk_s[:].rearrange("p (w k) -> p w k", w=12),
                    mybir.AxisListType.X, Alu.max,
                )

                # ---- chunk parity: gather even-chunk pair, pick element ----
                i2_s = smpool.tile([C, 12], f32, tag="i2")
                nc.vector.tensor_scalar(
                    i2_s[:], kp1_s[:], 512.0, -512.0, Alu.mult, Alu.add
                )
                nc.vector.tensor_add(i2_s[:], i2_s[:], s2_s[:])
                i2u_s = smpool.tile([C, 12], dt.int16, tag="i2u")
                nc.vector.tensor_copy(i2u_s[:], i2_s[:])
                g2_s = midpool.tile([C, 384], f16, tag="g2")
                nc.gpsimd.ap_gather(
                    g2_s[:], sbe[:], i2u_s[:],
                    channels=C, num_elems=4096, d=2, num_idxs=192,
                )
                g2m_s = midpool.tile([C, 384], f16, tag="g2m")
                nc.vector.tensor_mul(g2m_s[:], g2_s[:], dsel_s[:, 0:384])
                dpair_s = smpool.tile([C, 24], f32, tag="dpair")
                nc.vector.tensor_reduce(
                    dpair_s[:],
                    g2m_s[:].rearrange("p (w b pr) -> p w pr b", w=12, b=16),
                    mybir.AxisListType.X, Alu.add,
                )
                evd_s = smpool.tile([C, 12], f32, tag="evd")
                nc.vector.tensor_sub(
                    evd_s[:], dpair_s[:, 1::2], dpair_s[:, 0::2]
                )
                ev_s = smpool.tile([C, 12], f32, tag="ev")
                nc.vector.scalar_tensor_tensor(
                    ev_s[:], pb_s[:], 1.0, evd_s[:], Alu.mult, Alu.mult
                )
                nc.vector.tensor_add(ev_s[:], ev_s[:], dpair_s[:, 0::2])
                evq_s = smpool.tile([C, 12], f32, tag="evq")
                nc.vector.tensor_scalar(
                    evq_s[:], ev_s[:], LQ_SCALE, LQ_BIAS, Alu.mult, Alu.add
                )
                par_s = smpool.tile([C, 12], f32, tag="par")
                nc.vector.tensor_tensor(
                    par_s[:], evq_s[:], g0_s[:], Alu.not_equal
                )

                # ---- j (global winner index) ----
                j0_s = smpool.tile([C, 12], f32, tag="j0")
                nc.vector.scalar_tensor_tensor(
                    j0_s[:], par_s[:], 1024.0, s_sl[:], Alu.mult, Alu.add
                )
                jk_s = smpool.tile([C, 12], f32, tag="jk")
                nc.vector.tensor_scalar(
                    jk_s[:], kp1_s[:], 2048.0, -2048.0, Alu.mult, Alu.add
                )
                j_s = smpool.tile([C, 12], f32, tag="j")
                nc.vector.tensor_add(j_s[:], j0_s[:], jk_s[:])

                # ---- pair idx gp = floor(j/2), pair parity; padded to 16 ----
                gp_s = smpool.tile([C, 16], f32, tag="gp")
                nc.vector.memset(gp_s[:], 0.0)
                nc.vector.tensor_scalar(
                    gp_s[:, 0:12], j_s[:], 0.5, -0.25, Alu.mult, Alu.add
                )
                nc.vector.tensor_scalar(
                    gp_s[:, 0:12], gp_s[:, 0:12], MAGIC, MAGIC, Alu.add, Alu.subtract
                )
                gp_i = smpool.tile([C, 16], dt.int16, tag="gpi")
                nc.vector.tensor_copy(gp_i[:], gp_s[:])
                par2_s = smpool.tile([C, 16], f32, tag="par2")
                nc.vector.memset(par2_s[:], 0.0)
                nc.vector.scalar_tensor_tensor(
                    par2_s[:, 0:12], gp_s[:, 0:12], -2.0, j_s[:], Alu.mult, Alu.add
                )
                wn_s = smpool.tile([C, 16], f32, tag="wn")
                nc.vector.memset(wn_s[:], 0.0)
                nc.vector.tensor_scalar(
                    wn_s[:, 0:12], expv_s[:], rden_s[:], None, Alu.mult
                )
                wnE_s = smpool.tile([C, 16], f32, tag="wnE")
                nc.vector.scalar_tensor_tensor(
                    wnE_s[:], par2_s[:], -1.0, wn_s[:], Alu.mult, Alu.mult
                )
                nc.vector.tensor_add(wnE_s[:], wnE_s[:], wn_s[:])
                wnO_s = smpool.tile([C, 16], f32, tag="wnO")
                nc.vector.tensor_mul(wnO_s[:], wn_s[:], par2_s[:])

                # ---- gather V pairs (per-head core lists) ----
                g_s = smpool.tile([C, 512], bf16, tag="G")
                nc.gpsimd.ap_gather(
                    g_s[:], vt_s[:], gp_i[:],
                    channels=C, num_elems=NJ // 2, d=2, num_idxs=256,
                )

                # ---- weights -> [(h,d), (i,q,parity)] via headrep matmul ----
                wEb = (
                    wnE_s[:].rearrange("p (i o) -> p i o", o=1)
                    .to_broadcast([C, NQ, 32])
                )
                wOb = (
                    wnO_s[:].rearrange("p (i o) -> p i o", o=1)
                    .to_broadcast([C, NQ, 32])
                )
                tmpE = midpool.tile([C, 512], f16, tag="tmpE")
                nc.vector.tensor_mul(
                    tmpE[:].rearrange("p (i s) -> p i s", s=32),
                    wEb,
                    me_s[:].rearrange("p (i s) -> p i s", s=32),
                )
                wsc = midpool.tile([C, 512], f16, tag="wsc")
                nc.vector.tensor_mul(
                    wsc[:].rearrange("p (i s) -> p i s", s=32),
                    wOb,
                    mo_s[:].rearrange("p (i s) -> p i s", s=32),
                )
                nc.vector.tensor_add(wsc[:], wsc[:], tmpE[:])
                wb_ps = ps_m.tile([C, 512], f32, tag="misc")
                nc.tensor.matmul(wb_ps[:], hrep_s[:], wsc[:])
                wb_s = smpool.tile([C, 512], bf16, tag="wb")
                nc.scalar.copy(wb_s[:], wb_ps[:])

                gw_s = midpool.tile([C, 512], f32, tag="gw")
                nc.vector.tensor_mul(gw_s[:], g_s[:], wb_s[:])
                with nc.allow_low_precision(reason="pvt4 f16 out, values tiny"):
                    nc.vector.tensor_reduce(
                        pvt4_s[:, b::BPC],
                        gw_s[:].rearrange("p (i q r) -> p q i r", q=NQ, r=2),
                        mybir.AxisListType.XY,
                        Alu.add,
                    )

            # ---- final projections ----
            o1_ps = ps_m.tile([C, 512], f32, tag="misc")
            for qi in range(NQ):
                nc.tensor.matmul(
                    o1_ps[:, 0:BPC],
                    wjwp_s[:, qi * C:(qi + 1) * C],
                    pvt4_s[:, qi * BPC:(qi + 1) * BPC],
                    start=(qi == 0),
                    stop=(qi == NQ - 1),
                )
            o2_s = smpool.tile([C, BPC], f32, tag="o2")
            nc.vector.tensor_add(o2_s[:], o1_ps[:, 0:BPC], xT_s[:])
            o3_ps = ps_m.tile([C, 512], f32, tag="misc")
            nc.tensor.matmul(o3_ps[0:BPC, 0:C], o2_s[:], wp_s[:])
            o4_s = smpool.tile([BPC, C], f32, tag="o4")
            nc.vector.tensor_add(o4_s[:], o3_ps[0:BPC, 0:C], bp4_s[:])
            nc.sync.dma_start(out_d[:], o4_s[:])

    nc.compile()
    return nc


def _host_prep(inputs):
    x = np.asarray(inputs["x"], dtype=np.float32)
    complement = np.asarray(inputs["complement"], np.float32)
    Wq = np.asarray(inputs["Wq"], np.float32)
    Wkv = np.asarray(inputs["Wkv"], np.float32)
    Wjw = np.asarray(inputs["Wjw"], np.float32)
    Wp = np.asarray(inputs["Wp"], np.float32)
    bp = np.asarray(inputs["bp"], np.float32)

    wkT = np.empty((C, 8 * C), np.float32)
    wv = np.empty((C, 8 * C), np.float32)
    for e in range(8):
        wkT[:, e * C:(e + 1) * C] = Wkv[:, e * 256: e * 256 + 128].T
        wv[:, e * C:(e + 1) * C] = Wkv[:, e * 256 + 128: e * 256 + 256]
    wjwp = (
        Wjw.reshape(H, NQ, HD, C).transpose(1, 0, 2, 3).reshape(NQ, C, C)
        .transpose(1, 0, 2).reshape(C, NQ * C)
    )
    bp4 = np.tile(bp.reshape(1, C), (BPC, 1)).astype(np.float32)
    hrep = np.kron(np.eye(H, dtype=np.float32), np.ones((HD, HD), np.float32))
    s_idx = np.tile(np.arange(32).reshape(1, 1, 32), (C, NQ, 1))
    p_idx = (np.arange(C) % NQ).reshape(C, 1, 1)
    me = (s_idx == 2 * p_idx).astype(np.float32).reshape(C, 512)
    mo = (s_idx == 2 * p_idx + 1).astype(np.float32).reshape(C, 512)
    srow = np.tile(np.arange(1024, dtype=np.float32).reshape(1, 1024), (C, 1))
    krow = np.tile(
        (np.tile(np.arange(8, dtype=np.float32), 12) * 512).reshape(1, 96), (C, 1)
    )
    kval = np.repeat(
        np.tile(
            (np.tile(np.arange(8, dtype=np.float32), 12) + 1).reshape(1, 96), (C, 1)
        ), 2, axis=1,
    )
    dsel1 = np.zeros((C, 1536), np.float16)
    for p in range(C):
        dsel1[p, (np.arange(96) * 16 + p % 16)] = 1.0
    dsel = np.repeat(dsel1, 2, axis=1)

    shared = dict(
        wq=np.ascontiguousarray(Wq.astype(np.float16)),
        wkT=np.ascontiguousarray(wkT.astype(np.float16)),
        wv=np.ascontiguousarray(wv.astype(np.float16)),
        wjwp=np.ascontiguousarray(wjwp.astype(np.float16)),
        wp=np.ascontiguousarray(Wp),
        bp4=bp4,
        hrep=np.ascontiguousarray(hrep.astype(np.float16)),
        me=np.ascontiguousarray(me.astype(np.float16)),
        mo=np.ascontiguousarray(mo.astype(np.float16)),
        srow=srow.astype(np.float16),
        krow=np.ascontiguousarray(krow.astype(np.float16)),
        kval=np.ascontiguousarray(kval.astype(np.float16)),
        dsel=dsel,
    )

    in_maps = []
    for core in range(CORES):
        bs = range(core * BPC, (core + 1) * BPC)
        comp = np.stack(
            [
                np.concatenate([x[bb].reshape(1, C), complement[bb]], axis=0)
                for bb in bs
            ]
        ).astype(np.float32)
        compT = comp.transpose(0, 2, 1)
        m = dict(shared)
        m["c16h"] = np.ascontiguousarray(compT.astype(np.float16))
        xTc = np.ascontiguousarray(x[list(bs)].reshape(BPC, C).T)
        m["xT"] = xTc
        m["xT16"] = xTc.astype(np.float16)
        in_maps.append(m)
    return in_maps


def kernel(**inputs):
    from concourse.bass_utils import run_bass_kernel_spmd

    if "prog" not in _prog_cache:
        _prog_cache["prog"] = _build_program()
    nc = _prog_cache["prog"]

    in_maps = _host_prep(inputs)
    res = run_bass_kernel_spmd(nc, in_maps, core_ids=list(range(CORES)))
    out = np.empty((B, 1, C), np.float32)
    for core in range(CORES):
        o = res.results[core]["out"]
        for i in range(BPC):
            out[core * BPC + i, 0, :] = o[i]
    return out


if __name__ == "__main__":
    d = np.load("/root/problem/inputs_cache.npz")
    inputs = {k: d[k] for k in d.files if k != "ref_out"}
    ref = d["ref_out"]
    got = kernel(**inputs)
    err = np.abs(got - ref)
    print("absmax err:", err.max())
    print("Relative error:", err.max() / np.abs(ref).max())
    print("rel l2:", np.linalg.norm(got - ref) / np.linalg.norm(ref))


# revision 15
# speedup vs baseline: 1.9477x; 1.9366x over previous
"""Trainium2 Bass kernel for nn_MultiHeadCrossAttention (B=32, Nc=2048, H=8, topk=12).

kernel(**inputs) takes FULL inputs, returns FULL output [32, 1, 128].
Batch is sharded 4-per-core across 8 NeuronCores (data parallel, no collectives).

Per-batch device algorithm (rows=(h,q) 128 wide, j = e*2048+nc in [0,16384)):
  comp_T via PE transposes, split into fp16 hi/lo
  A_e[c,row]  = WkT_e.T @ Qbd (fp32), split into fp16 hi/lo
  S_e[row,nc] = Ah.T@Ch + Ah.T@Cl + Al.T@Ch   (3x fp16 matmuls ~ fp32 exact)
  VT_e[hd,nc] = Wv_e.T @ Ch -> VT [128,16384] bf16
  per-chunk(1024) top8 (DVE max8) -> cand [128,128]
  per-chunk max_index -> local indices
  exact global top-12 marking via max8/match_replace rounds on cand
  pack (global_idx*1024 + quantized_value), extract winners via max8
  weights = exp(value)/sum   (selection exact; weight quantization ~0.4%)
  G = ap_gather(VT pairs, winner idx/2; 16-partition cores align with heads)
  PV^T[(h,d),q] = sum w*G  (headrep matmul broadcasts weights, parity split)
  out = (PV flat @ WjwP) + x;  out = out @ Wp + bp
"""

import sys
import numpy as np

for p in ("/opt/trn_rl_repo",):
    if p not in sys.path:
        sys.path.insert(0, p)

import ml_dtypes

B, CORES, BPC = 32, 8, 4
H, HD, NQ, TK, C, NC = 8, 16, 16, 12, 128, 2048
NJ = 8 * NC            # 16384
CHUNK = 1024
NCH = NJ // CHUNK      # 16
NCAND = NCH * 8        # 128
NEG = -1e30
MAGIC = 12582912.0     # 2**23 + 2**22: add/sub rounds fp32 to nearest int

_prog_cache = {}


def _build_program():
    import concourse.bass as bass
    import concourse.mybir as mybir
    import concourse.tile as tile
    from concourse import bacc
    from concourse import library_config

    dt = mybir.dt
    Alu = mybir.AluOpType
    f32, f16, bf16 = dt.float32, dt.float16, dt.bfloat16
    nc = bacc.Bacc("TRN2", target_bir_lowering=False)

    comphT_d = nc.dram_tensor("comphT", [BPC, C, NC], f16, kind="ExternalInput")
    complT_d = nc.dram_tensor("complT", [BPC, C, NC], f16, kind="ExternalInput")
    xT_d = nc.dram_tensor("xT", [C, BPC], f32, kind="ExternalInput")
    wq_d = nc.dram_tensor("wq", [C, 2048], f32, kind="ExternalInput")
    wkT_d = nc.dram_tensor("wkT", [C, 8 * C], f32, kind="ExternalInput")
    wv_d = nc.dram_tensor("wv", [C, 8 * C], f16, kind="ExternalInput")
    wjwp_d = nc.dram_tensor("wjwp", [C, NQ * C], f32, kind="ExternalInput")
    wp_d = nc.dram_tensor("wp", [C, C], f32, kind="ExternalInput")
    bp4_d = nc.dram_tensor("bp4", [BPC, C], f32, kind="ExternalInput")
    hrep_d = nc.dram_tensor("hrep", [C, C], f32, kind="ExternalInput")
    choff_d = nc.dram_tensor("choff", [C, NCAND], f32, kind="ExternalInput")
    me_d = nc.dram_tensor("me", [C, 512], f32, kind="ExternalInput")
    mo_d = nc.dram_tensor("mo", [C, 512], f32, kind="ExternalInput")
    out_d = nc.dram_tensor("out", [BPC, C], f32, kind="ExternalOutput")

    with tile.TileContext(nc) as tc:
        nc.gpsimd.load_library(library_config.ap_gather)
        with (
            tc.tile_pool(name="weights", bufs=1) as wpool,
            tc.tile_pool(name="compt", bufs=2) as ctpool,
            tc.tile_pool(name="bigS", bufs=2) as spool,
            tc.tile_pool(name="bigV", bufs=1) as vpool,
            tc.tile_pool(name="small", bufs=1) as smpool,
            tc.tile_pool(name="dmain", bufs=4) as scpool,
            tc.tile_pool(name="ps_big", bufs=3, space="PSUM") as ps_big,
            tc.tile_pool(name="ps_a", bufs=1, space="PSUM") as ps_a,
            tc.tile_pool(name="ps_misc", bufs=1, space="PSUM") as ps_m,
        ):
            # ---- weights / constants resident ----
            wq_s = wpool.tile([C, 2048], f32)
            nc.sync.dma_start(wq_s[:], wq_d[:])
            wkT_s = wpool.tile([C, 8 * C], f32)
            nc.sync.dma_start(wkT_s[:], wkT_d[:])
            wv_s = wpool.tile([C, 8 * C], f16)
            nc.sync.dma_start(wv_s[:], wv_d[:])
            wjwp_s = wpool.tile([C, NQ * C], f32)
            nc.sync.dma_start(wjwp_s[:], wjwp_d[:])
            wp_s = wpool.tile([C, C], f32)
            nc.sync.dma_start(wp_s[:], wp_d[:])
            bp4_s = wpool.tile([BPC, C], f32)
            nc.sync.dma_start(bp4_s[:], bp4_d[:])
            hrep_s = wpool.tile([C, C], f32)
            nc.sync.dma_start(hrep_s[:], hrep_d[:])
            choff_s = wpool.tile([C, NCAND], f32)
            nc.sync.dma_start(choff_s[:], choff_d[:])
            me_s = wpool.tile([C, 512], f32)
            nc.sync.dma_start(me_s[:], me_d[:])
            mo_s = wpool.tile([C, 512], f32)
            nc.sync.dma_start(mo_s[:], mo_d[:])
            xT_s = wpool.tile([C, BPC], f32)
            nc.sync.dma_start(xT_s[:], xT_d[:])

            # ---- Q projection, all 4 batches: QT [(h,d), (q,b)] ----
            qt_ps = ps_m.tile([C, 512], f32, tag="misc")
            for q in range(NQ):
                nc.tensor.matmul(
                    qt_ps[:, q * BPC:(q + 1) * BPC],
                    wq_s[:, q * C:(q + 1) * C],
                    xT_s[:],
                )
            qt_s = wpool.tile([C, NQ * BPC], f32)
            nc.scalar.copy(qt_s[:], qt_ps[:, : NQ * BPC])

            pvt4_s = wpool.tile([C, NQ * BPC], f32)   # [(h,d), (q,b)]

            for b in range(BPC):
                # ---- comp_T fp16 hi/lo: host-transposed, DMA straight in ----
                c16h = ctpool.tile([C, NC], f16, tag="c16h")
                nc.sync.dma_start(c16h[:], comphT_d[b])

                # ---- Qbd block-diag with 0.25 scale ----
                qfull_s = smpool.tile([C, C], f32, tag="qfull")
                qsl = (
                    qt_s[:, b::BPC]
                    .rearrange("p (o q) -> p o q", o=1)
                    .to_broadcast([C, H, NQ])
                )
                nc.vector.tensor_scalar(
                    qfull_s[:].rearrange("p (o q) -> p o q", o=H),
                    qsl, 0.25, None, Alu.mult,
                )
                qbd_s = smpool.tile([C, C], f32, tag="qbd")
                nc.vector.tensor_mul(qbd_s[:], qfull_s[:], hrep_s[:])

                # ---- A_e [c,row] fp32 -> fp16 hi/lo ----
                a16h = smpool.tile([C, 8 * C], f16, tag="a16h")
                for half in range(2):
                    a_ps = ps_a.tile([C, 512], f32, tag="a")
                    for i in range(4):
                        e = half * 4 + i
                        nc.tensor.matmul(
                            a_ps[:, i * C:(i + 1) * C],
                            wkT_s[:, e * C:(e + 1) * C],
                            qbd_s[:],
                        )
                    dh = a16h[:, half * 512:(half + 1) * 512]
                    nc.scalar.copy(dh, a_ps[:])

                # ---- S = Ah.Ch + Ah.Cl + Al.Ch (fp16 x3) ----
                sh0 = spool.tile([C, NJ // 2], f32, tag="sh")
                sh1 = spool.tile([C, NJ // 2], f32, tag="sh")
                s_half = [sh0, sh1]
                for e in range(8):
                    ah = a16h[:, e * C:(e + 1) * C]
                    for half in range(2):
                        s_ps = ps_big.tile([C, 1024], f32, tag="big")
                        # single fp16 term: quantization budget covers it
                        for lhs, cc, st, sp in (
                            (ah, c16h, True, True),
                        ):
                            for n in range(2):
                                col = half * 1024 + n * 512
                                nc.tensor.matmul(
                                    s_ps[:, n * 512:(n + 1) * 512],
                                    lhs, cc[:, col:col + 512],
                                    start=st, stop=sp,
                                )
                        sh = s_half[(e * 2 + half) // 8]
                        off = ((e * 2 + half) % 8) * 1024
                        nc.scalar.copy(sh[:, off:off + 1024], s_ps[:])

                # ---- V^T (fp16 inputs, bf16 out) ----
                vt_s = vpool.tile([C, NJ], bf16, tag="VT")
                for e in range(8):
                    for half in range(2):
                        v_ps = ps_big.tile([C, 1024], f32, tag="big")
                        for n in range(2):
                            col = half * 1024 + n * 512
                            nc.tensor.matmul(
                                v_ps[:, n * 512:(n + 1) * 512],
                                wv_s[:, e * C:(e + 1) * C],
                                c16h[:, col:col + 512],
                            )
                        nc.scalar.copy(
                            vt_s[:, e * NC + half * 1024: e * NC + (half + 1) * 1024],
                            v_ps[:],
                        )

                # ---- per-chunk top8 + local indices (chunk=1024) ----
                cand_s = smpool.tile([C, NCAND], f32, tag="cand")
                li_s = smpool.tile([C, NCAND], dt.uint16, tag="li")
                for ch in range(NCH):
                    sh = s_half[ch // 8]
                    sl = sh[:, (ch % 8) * CHUNK:(ch % 8 + 1) * CHUNK]
                    nc.vector.max(cand_s[:, ch * 8:(ch + 1) * 8], sl)
                for ch in range(NCH):
                    sh = s_half[ch // 8]
                    sl = sh[:, (ch % 8) * CHUNK:(ch % 8 + 1) * CHUNK]
                    nc.vector.max_index(
                        li_s[:, ch * 8:(ch + 1) * 8],
                        cand_s[:, ch * 8:(ch + 1) * 8],
                        sl,
                    )

                # ---- exact top-12 marking on cand ----
                t8a = smpool.tile([C, 8], f32, tag="t8a")
                nc.vector.max(t8a[:], cand_s[:])
                c2 = smpool.tile([C, NCAND], f32, tag="c2")
                nc.vector.match_replace(c2[:], t8a[:], cand_s[:], NEG)
                t8b = smpool.tile([C, 8], f32, tag="t8b")
                nc.vector.max(t8b[:], c2[:])
                nx4 = smpool.tile([C, 8], f32, tag="nx4")
                nc.vector.memset(nx4[:], 1e30)
                nc.vector.tensor_copy(nx4[:, 0:4], t8b[:, 0:4])
                rr = smpool.tile([C, NCAND], f32, tag="rr")
                nc.vector.match_replace(rr[:], nx4[:], c2[:], NEG)
                mask12 = smpool.tile([C, NCAND], f32, tag="mask12")
                nc.vector.tensor_scalar(mask12[:], rr[:], -1e29, None, Alu.is_le)

                # ---- pack global_idx*1024 + q10(value); mask; extract ----
                lif = smpool.tile([C, NCAND], f32, tag="lif")
                nc.vector.tensor_copy(lif[:], li_s[:])
                gfl = smpool.tile([C, NCAND], f32, tag="gfl")
                nc.vector.tensor_scalar(gfl[:], lif[:], 1024.0, None, Alu.mult)
                nc.vector.tensor_add(gfl[:], gfl[:], choff_s[:])
                q10 = smpool.tile([C, NCAND], f32, tag="q10")
                nc.vector.tensor_scalar(
                    q10[:], cand_s[:], 4.0, 128.0, Alu.add, Alu.mult
                )
                nc.vector.tensor_scalar(
                    q10[:], q10[:], 1023.0, 1.0, Alu.min, Alu.max
                )
                pm = smpool.tile([C, NCAND], f32, tag="pm")
                nc.vector.tensor_add(pm[:], gfl[:], q10[:])
                nc.vector.tensor_mul(pm[:], pm[:], mask12[:])

                pw = smpool.tile([C, 16], f32, tag="pw")
                nc.vector.max(pw[:, 0:8], pm[:])
                pm2 = smpool.tile([C, NCAND], f32, tag="pm2")
                nc.vector.match_replace(pm2[:], pw[:, 0:8], pm[:], 0.0)
                nc.vector.max(pw[:, 8:16], pm2[:])

                # ---- decode winners: gidx + value -> weights ----
                gidxf = smpool.tile([C, 16], f32, tag="gidxf")
                nc.vector.tensor_scalar(
                    gidxf[:], pw[:], 1.0 / 1024.0, -0.5, Alu.mult, Alu.add
                )
                nc.vector.tensor_scalar(
                    gidxf[:], gidxf[:], MAGIC, MAGIC, Alu.add, Alu.subtract
                )
                vv = smpool.tile([C, 16], f32, tag="vv")
                nc.vector.tensor_scalar(vv[:], gidxf[:], -1024.0, None, Alu.mult)
                nc.vector.tensor_add(vv[:], vv[:], pw[:])
                nc.vector.tensor_scalar(
                    vv[:], vv[:], 1.0 / 128.0, -4.0, Alu.mult, Alu.add
                )
                expv = smpool.tile([C, 16], f32, tag="expv")
                nc.scalar.activation(
                    expv[:], vv[:], mybir.ActivationFunctionType.Exp
                )
                wmask = smpool.tile([C, 16], f32, tag="wmask")
                nc.vector.tensor_scalar(wmask[:], pw[:], 0.5, None, Alu.is_ge)
                wgt = smpool.tile([C, 16], f32, tag="wgt")
                nc.vector.tensor_mul(wgt[:], expv[:], wmask[:])
                den = smpool.tile([C, 1], f32, tag="den")
                nc.vector.tensor_reduce(
                    den[:], wgt[:], mybir.AxisListType.X, Alu.add
                )
                rden = smpool.tile([C, 1], f32, tag="rden")
                nc.vector.reciprocal(rden[:], den[:])
                wn = smpool.tile([C, 16], f32, tag="wn")
                nc.vector.tensor_scalar(wn[:], wgt[:], rden[:], None, Alu.mult)

                # ---- pair index (bf16 gather needs 4B granules: d=2) ----
                gp = smpool.tile([C, 16], f32, tag="gp")
                nc.vector.tensor_scalar(
                    gp[:], gidxf[:], 0.5, -0.25, Alu.mult, Alu.add
                )
                nc.vector.tensor_scalar(
                    gp[:], gp[:], MAGIC, MAGIC, Alu.add, Alu.subtract
                )
                gp_i = smpool.tile([C, 16], dt.int16, tag="gpi")
                nc.vector.tensor_copy(gp_i[:], gp[:])
                par = smpool.tile([C, 16], f32, tag="par")
                nc.vector.tensor_scalar(par[:], gp[:], -2.0, None, Alu.mult)
                nc.vector.tensor_add(par[:], par[:], gidxf[:])
                parc = smpool.tile([C, 16], f32, tag="parc")
                nc.vector.tensor_scalar(
                    parc[:], par[:], -1.0, 1.0, Alu.mult, Alu.add
                )
                wnE = smpool.tile([C, 16], f32, tag="wnE")
                nc.vector.tensor_mul(wnE[:], wn[:], parc[:])
                wnO = smpool.tile([C, 16], f32, tag="wnO")
                nc.vector.tensor_mul(wnO[:], wn[:], par[:])

                # ---- gather V pairs (per-head core lists) ----
                g_s = smpool.tile([C, 512], bf16, tag="G")
                nc.gpsimd.ap_gather(
                    g_s[:], vt_s[:], gp_i[:],
                    channels=C, num_elems=NJ // 2, d=2, num_idxs=256,
                )

                # ---- weights -> [(h,d), (i,q,parity)] via headrep matmul ----
                wEb = (
                    wnE[:].rearrange("p (i o) -> p i o", o=1)
                    .to_broadcast([C, NQ, 32])
                )
                wOb = (
                    wnO[:].rearrange("p (i o) -> p i o", o=1)
                    .to_broadcast([C, NQ, 32])
                )
                tmpE = smpool.tile([C, 512], f32, tag="tmpE")
                nc.vector.tensor_mul(
                    tmpE[:].rearrange("p (i s) -> p i s", s=32),
                    wEb,
                    me_s[:].rearrange("p (i s) -> p i s", s=32),
                )
                wsc = smpool.tile([C, 512], f32, tag="wsc")
                nc.vector.tensor_mul(
                    wsc[:].rearrange("p (i s) -> p i s", s=32),
                    wOb,
                    mo_s[:].rearrange("p (i s) -> p i s", s=32),
                )
                nc.vector.tensor_add(wsc[:], wsc[:], tmpE[:])
                wb_ps = ps_m.tile([C, 512], f32, tag="misc")
                nc.tensor.matmul(wb_ps[:], hrep_s[:], wsc[:])
                wb_s = smpool.tile([C, 512], bf16, tag="wb")
                nc.scalar.copy(wb_s[:], wb_ps[:])

                gw = smpool.tile([C, 512], f32, tag="gw")
                nc.vector.tensor_mul(gw[:], g_s[:], wb_s[:])
                # reduce over (i, parity), keep q: write PV^T into (q,b) cols
                nc.vector.tensor_reduce(
                    pvt4_s[:, b::BPC],
                    gw[:].rearrange("p (i q r) -> p q i r", q=NQ, r=2),
                    mybir.AxisListType.XY,
                    Alu.add,
                )

            # ---- final projections for all 4 batches ----
            o1_ps = ps_m.tile([C, 512], f32, tag="misc")
            for q in range(NQ):
                nc.tensor.matmul(
                    o1_ps[:, 0:BPC],
                    wjwp_s[:, q * C:(q + 1) * C],
                    pvt4_s[:, q * BPC:(q + 1) * BPC],
                    start=(q == 0),
                    stop=(q == NQ - 1),
                )
            o2_s = smpool.tile([C, BPC], f32, tag="o2")
            nc.vector.tensor_add(o2_s[:], o1_ps[:, 0:BPC], xT_s[:])
            o3_ps = ps_m.tile([C, 512], f32, tag="misc")
            nc.tensor.matmul(o3_ps[0:BPC, 0:C], o2_s[:], wp_s[:])
            o4_s = smpool.tile([BPC, C], f32, tag="o4")
            nc.vector.tensor_add(o4_s[:], o3_ps[0:BPC, 0:C], bp4_s[:])
            nc.sync.dma_start(out_d[:], o4_s[:])

    nc.compile()
    return nc


def _host_prep(inputs):
    x = np.asarray(inputs["x"], dtype=np.float32)              # [32, 1, 128]
    complement = np.asarray(inputs["complement"], np.float32)  # [32, 2047, 128]
    Wq = np.asarray(inputs["Wq"], np.float32)
    Wkv = np.asarray(inputs["Wkv"], np.float32)
    Wjw = np.asarray(inputs["Wjw"], np.float32)
    Wp = np.asarray(inputs["Wp"], np.float32)
    bp = np.asarray(inputs["bp"], np.float32)

    wkT = np.empty((C, 8 * C), np.float32)
    wv = np.empty((C, 8 * C), np.float32)
    for e in range(8):
        wkT[:, e * C:(e + 1) * C] = Wkv[:, e * 256: e * 256 + 128].T
        wv[:, e * C:(e + 1) * C] = Wkv[:, e * 256 + 128: e * 256 + 256]
    wv = wv.astype(np.float16)
    # Wjw rows are (h,q,d); per-q slice with rows (h,d)
    wjwp = (
        Wjw.reshape(H, NQ, HD, C).transpose(1, 0, 2, 3).reshape(NQ, C, C)
        .transpose(1, 0, 2).reshape(C, NQ * C)
    )
    bp4 = np.tile(bp.reshape(1, C), (BPC, 1)).astype(np.float32)
    hrep = np.kron(np.eye(H, dtype=np.float32), np.ones((HD, HD), np.float32))
    choffrow = ((np.arange(NCAND) // 8) * (CHUNK * 1024)).astype(np.float32)
    choff = np.tile(choffrow.reshape(1, NCAND), (C, 1))
    s_idx = np.tile(np.arange(32).reshape(1, 1, 32), (C, NQ, 1))
    p_idx = (np.arange(C) % NQ).reshape(C, 1, 1)
    me = (s_idx == 2 * p_idx).astype(np.float32).reshape(C, 512)
    mo = (s_idx == 2 * p_idx + 1).astype(np.float32).reshape(C, 512)

    shared = dict(
        wq=np.ascontiguousarray(Wq),
        wkT=np.ascontiguousarray(wkT),
        wv=np.ascontiguousarray(wv),
        wjwp=np.ascontiguousarray(wjwp),
        wp=np.ascontiguousarray(Wp),
        bp4=bp4,
        hrep=np.ascontiguousarray(hrep),
        choff=np.ascontiguousarray(choff),
        me=np.ascontiguousarray(me),
        mo=np.ascontiguousarray(mo),
    )

    in_maps = []
    for core in range(CORES):
        bs = range(core * BPC, (core + 1) * BPC)
        comp = np.stack(
            [
                np.concatenate([x[b].reshape(1, C), complement[b]], axis=0)
                for b in bs
            ]
        ).astype(np.float32)
        compT = comp.transpose(0, 2, 1)          # [BPC, C, NC]
        comphT = compT.astype(np.float16)
        complT = (compT - comphT.astype(np.float32)).astype(np.float16)
        xT = np.ascontiguousarray(x[list(bs)].reshape(BPC, C).T)
        m = dict(shared)
        m["comphT"] = np.ascontiguousarray(comphT)
        m["complT"] = np.ascontiguousarray(complT)
        m["xT"] = xT
        in_maps.append(m)
    return in_maps


def kernel(**inputs):
    from concourse.bass_utils import run_bass_kernel_spmd

    if "prog" not in _prog_cache:
        _prog_cache["prog"] = _build_program()
    nc = _prog_cache["prog"]

    in_maps = _host_prep(inputs)
    res = run_bass_kernel_spmd(nc, in_maps, core_ids=list(range(CORES)))
    out = np.empty((B, 1, C), np.float32)
    for core in range(CORES):
        o = res.results[core]["out"]
        for i in range(BPC):
            out[core * BPC + i, 0, :] = o[i]
    return out


if __name__ == "__main__":
    d = np.load("/root/problem/inputs_cache.npz")
    inputs = {k: d[k] for k in d.files}
    got = kernel(**inputs)
    print("kernel output:", got.shape, got.dtype, np.abs(got).max())

